# revision 9
# baseline (speedup 1.0000x reference)
"""Trainium2 Bass/Tile kernel for a dense transformer block.

Math (per batch element b, T=16 tokens, C=512, H=8 heads, D=64):
    h  = LN(x; ln1_g, ln1_b)
    q,k,v = per-head projections of h
    att = causal-softmax(q k^T / sqrt(D)); o = att v (heads concatenated)
    y  = o @ w_proj + b_proj + x
    f  = relu(LN(y; ln2_g, ln2_b) @ w1 + b1) @ w2 + b2
    out = f + y

Distribution: pure data parallel over the batch dim (4096) across 8
NeuronCores; weights replicated; no collectives.

v3: fully cyclic software pipeline over 128-token tiles. Each tile
iteration emits the SAME work mix (v+logits+softmax chain for tile ti,
o-matmuls for ti-1, proj for ti-2, LN2 for ti-3, plus a steady drip of
deferred MLP chunks), across group boundaries, so the PE never sees the
group-tail serial chain that caused HAM clock-gate oscillation (the
~330us of K=4/8 throttling in v2).
  * logits are row-tiled (tile_position, K=64, 2 heads concurrent in
    the PE array) so k^T needs no zero-padded copies: one eviction per
    k chunk, no gpsimd memsets.
  * c2 (MLP down-proj) emits token-major (lhsT = r slices, rhs = w2
    row-paired fp8 DR): no fT transposes, eviction fused with the
    +y residual add in one DVE op.
  * proj runs fp8 DoubleRow (oT evicted WS-scaled fp8).
  * causal mask is a multiplicative DVE op post-exp (stride-0
    broadcast over heads); att row-scale is one broadcast tensor_tensor.
  * q,k evicted WS-scaled fp8 (FWL on the logits weight loads).
"""

import sys

sys.path.insert(0, "/opt/trn_rl_repo")

import numpy as np
import ml_dtypes

import concourse.bass as bass
import concourse.tile as tile
from concourse import bacc, mybir
from concourse.bass import broadcast_tensor_aps
from concourse.bass_utils import run_bass_kernel_spmd

F32 = mybir.dt.float32
BF16 = mybir.dt.bfloat16
FP8 = mybir.dt.float8e4
DR = mybir.MatmulPerfMode.DoubleRow
AF = mybir.ActivationFunctionType
ALU = mybir.AluOpType

WS = 32.0

NCORES = 8
B, T, C, H, D = 4096, 16, 512, 8, 64
HD = H * D          # 512
M1 = 4 * C          # 2048
EPS = 1e-5
BL = B // NCORES    # 512 batch elems per core
NTOK_FULL = BL * T  # 8192 tokens per core
P = 128             # partitions
GT = 512            # tokens per group
KC = C // P         # 4 c-chunks
KM = M1 // P        # 16 hidden chunks

# deferred-MLP drain counts by tile phase (chunks for group g are created
# at ti=4g+6, i.e. phase 2; phase 0 is light because B1 runs there)
DRAINS = (2, 6, 6, 6)


def emit_block(ctx, tc, outs, ins, ntok):
    """Emit the transformer-block program. outs/ins: dicts of DRAM APs."""
    nc = tc.nc
    x_d = ins["x"]
    wqk_d = ins["wqk"]
    wv_d = ins["wv"]
    wp_d = ins["wp"]
    w1_d = ins["w1"]
    w2_d = ins["w2"]
    bqk_d = ins["bqk"]
    b1_d = ins["b1"]
    b2r_d = ins["b2r"]
    bpe_d = ins["bpe"]
    mask_d = ins["mask"]
    ident_d = ins["ident"]
    out_d = outs["out"]

    ngroups = ntok // GT
    ntiles = ntok // P
    assert ntok % GT == 0

    consts = ctx.enter_context(tc.tile_pool(name="consts", bufs=1))

    wqk = [consts.tile([P, 2 * 2 * HD], FP8, tag=f"wqk{m}", name=f"wqk{m}")
           for m in range(2)]
    wv = [consts.tile([P, 2 * HD], FP8, tag=f"wv{m}", name=f"wv{m}")
          for m in range(2)]
    wp = [consts.tile([P, 2 * C], FP8, tag=f"wp{m}", name=f"wp{m}")
          for m in range(2)]
    w1 = [consts.tile([P, 2 * M1], FP8, tag=f"w1{m}", name=f"w1{m}")
          for m in range(2)]
    w2 = [consts.tile([P, 2 * C], FP8, tag=f"w2{m}", name=f"w2{m}")
          for m in range(8)]
    for m in range(2):
        nc.sync.dma_start(wqk[m][:], wqk_d[m * P:(m + 1) * P, :])
        nc.sync.dma_start(wv[m][:], wv_d[m * P:(m + 1) * P, :])
        nc.sync.dma_start(wp[m][:], wp_d[m * P:(m + 1) * P, :])
        nc.sync.dma_start(w1[m][:], w1_d[m * P:(m + 1) * P, :])
    for m in range(8):
        nc.sync.dma_start(w2[m][:], w2_d[m * P:(m + 1) * P, :])

    bqk = consts.tile([P, 12], F32, tag="bqk", name="bqk")
    scm = consts.tile([P, 2], F32, tag="scm", name="scm")
    b1t = consts.tile([P, KM], F32, tag="b1t", name="b1t")
    nc.sync.dma_start(bqk[:], bqk_d[:, :])
    nc.sync.dma_start(scm[:], ins["scm"][:, :])
    nc.sync.dma_start(b1t[:], b1_d.rearrange("(j p) -> p j", p=P))

    bpe = consts.tile([1, C], BF16, tag="bpe", name="bpe")
    nc.sync.dma_start(bpe[:], bpe_d[:, :])
    b2r = consts.tile([1, C], BF16, tag="b2r", name="b2r")
    nc.sync.dma_start(b2r[:], b2r_d[:, :])
    mask = consts.tile([P, P], BF16, tag="mask", name="mask")
    nc.sync.dma_start(mask[:], mask_d[:, :])
    ident = consts.tile([P, P], BF16, tag="ident", name="ident")
    nc.sync.dma_start(ident[:], ident_d[:, :])
    ones1 = consts.tile([1, P], BF16, tag="ones1", name="ones1")
    nc.vector.memset(ones1[:], 1.0)
    epst = consts.tile([P, 1], F32, tag="epst", name="epst")
    nc.vector.memset(epst[:], EPS)

    # --- working pools ---
    p_x = ctx.enter_context(tc.tile_pool(name="p_x", bufs=12))
    p_h = ctx.enter_context(tc.tile_pool(name="p_h", bufs=3))
    p_hT = ctx.enter_context(tc.tile_pool(name="p_hT", bufs=2))
    p_qk = ctx.enter_context(tc.tile_pool(name="p_qk", bufs=24))
    p_v = ctx.enter_context(tc.tile_pool(name="p_v", bufs=6))
    p_S = ctx.enter_context(tc.tile_pool(name="p_S", bufs=4))
    p_att = ctx.enter_context(tc.tile_pool(name="p_att", bufs=4))
    p_attT = ctx.enter_context(tc.tile_pool(name="p_attT", bufs=4))
    p_oT = ctx.enter_context(tc.tile_pool(name="p_oT", bufs=6))
    p_y = ctx.enter_context(tc.tile_pool(name="p_y", bufs=10))
    p_h2T = ctx.enter_context(tc.tile_pool(name="p_h2T", bufs=3))
    p_r = ctx.enter_context(tc.tile_pool(name="p_r", bufs=20))
    p_out = ctx.enter_context(tc.tile_pool(name="p_out", bufs=4))
    p_st = ctx.enter_context(tc.tile_pool(name="p_st", bufs=10))

    ps_mm = ctx.enter_context(tc.tile_pool(name="ps_mm", bufs=4, space="PSUM"))
    ps_log = ctx.enter_context(tc.tile_pool(name="ps_log", bufs=2, space="PSUM"))
    ps_t = ctx.enter_context(tc.tile_pool(name="ps_t", bufs=2, space="PSUM"))

    # --- cross-iteration state, keyed by tile/group index ---
    xs = {}        # tj -> x tile
    hTs = {}       # g -> hT group tile [P, KC*GT]
    qTs = {}       # g -> (qT list, kT list)
    vs = {}        # tj -> v tile
    oTs = {}       # tj -> oT tile
    attTs = {}     # tj -> attT tile
    ys = {}        # tj -> y tile
    h2Ts = {}      # g -> h2T group tile
    pend_c = []

    def drain_c(n):
        for _ in range(min(n, len(pend_c))):
            pend_c.pop(0)()

    def layernorm(x_t, h_t):
        st = p_st.tile([P, 6], F32, tag="bn", name="bn")
        mv = p_st.tile([P, 2], F32, tag="mv", name="mv")
        nc.vector.bn_stats(st[:], x_t[:])
        nc.vector.bn_aggr(mv[:], st[:])
        lnv = p_st.tile([P, 1], F32, tag="lnv", name="lnv")
        rstd = p_st.tile([P, 1], F32, tag="rstd", name="rstd")
        nc.scalar.activation(lnv[:], mv[:, 1:2], AF.Ln, bias=epst[:])
        nc.scalar.activation(rstd[:], lnv[:], AF.Exp, scale=-0.5)
        nc.vector.tensor_scalar(
            out=h_t[:], in0=x_t[:],
            scalar1=mv[:, 0:1], scalar2=rstd[:],
            op0=ALU.subtract, op1=ALU.mult,
        )

    def stage_a_dma(g):
        for i in range(4):
            tj = 4 * g + i
            xt = p_x.tile([P, C], F32, tag="x", name="x")
            xs[tj] = xt
            nc.sync.dma_start(xt[:], x_d[tj * P:(tj + 1) * P, :])

    def stage_a_tile(g, i):
        """LN1 + feature-major transpose of tile (g, i) into hTs[g]."""
        if i == 0:
            hTs[g] = p_hT.tile([P, KC * GT], FP8, tag="hT", name="hT")
        hT3 = hTs[g][:].rearrange("p (c t) -> p c t", c=KC)
        ht = p_h.tile([P, C], BF16, tag="h", name="h")
        layernorm(xs[4 * g + i], ht)
        pst = ps_t.tile([P, KC * P], BF16, tag="pst", name="pst")
        for c in range(KC):
            nc.tensor.transpose(pst[:, c * P:(c + 1) * P],
                                ht[:, c * P:(c + 1) * P], ident[:])
        nc.vector.tensor_copy(hT3[:, :, i * P:(i + 1) * P], pst[:])

    def emit_b1(g):
        """q^T, k^T for group g (weight stationary, fp8-DR, WS-scaled fp8
        evictions). k is evicted twice per chunk with a per-partition 0/1
        scale mask so each copy holds one head's rows zero-padded (keeps
        the logits matmuls full-K=128 with no memsets)."""
        hT3 = hTs[g][:].rearrange("p (c t) -> p c t", c=KC)
        qT = [p_qk.tile([P, GT], FP8, tag="qk", name="qk") for _ in range(KC)]
        kTe = [p_qk.tile([P, GT], FP8, tag="qk", name="qk") for _ in range(KC)]
        kTo = [p_qk.tile([P, GT], FP8, tag="qk", name="qk") for _ in range(KC)]
        qTs[g] = (qT, kTe, kTo)
        for j in range(8):
            ps = ps_mm.tile([P, GT], F32, tag="mm", name="mm")
            for m in range(2):
                wqks = wqk[m][:].rearrange("p (c j) -> p c j", c=2)
                nc.tensor.matmul(
                    ps[:], wqks[:, :, j * P:(j + 1) * P],
                    hT3[:, 2 * m:2 * m + 2, :],
                    start=(m == 0), stop=(m == 1), perf_mode=DR,
                )
            if j < 4:
                nc.scalar.activation(qT[j][:], ps[:], AF.Identity,
                                     bias=bqk[:, j:j + 1])
            else:
                hp = j - 4
                nc.scalar.activation(kTe[hp][:], ps[:], AF.Identity,
                                     scale=scm[:, 0:1], bias=bqk[:, 4 + hp:5 + hp])
                nc.scalar.activation(kTo[hp][:], ps[:], AF.Identity,
                                     scale=scm[:, 1:2], bias=bqk[:, 8 + hp:9 + hp])

    def emit_v(tj):
        g, i = divmod(tj, 4)
        hT3 = hTs[g][:].rearrange("p (c t) -> p c t", c=KC)
        ps = ps_mm.tile([P, HD], F32, tag="mm", name="mm")
        for m in range(2):
            wvs = wv[m][:].rearrange("p (c d) -> p c d", c=2)
            nc.tensor.matmul(
                ps[:], hT3[:, 2 * m:2 * m + 2, i * P:(i + 1) * P],
                wvs[:],
                start=(m == 0), stop=(m == 1), perf_mode=DR,
            )
        vt = p_v.tile([P, HD], BF16, tag="v", name="v")
        vs[tj] = vt
        nc.scalar.activation(vt[:], ps[:], AF.Identity, scale=1.0 / WS)

    def emit_att(tj):
        """logits (row-tiled, K=64) + exp + mask + normalize + attT."""
        g, i = divmod(tj, 4)
        qT, kTe, kTo = qTs[g]
        sl = slice(i * P, (i + 1) * P)
        S = p_S.tile([P, H * P], BF16, tag="S", name="S")
        for half in range(2):
            ps_l = ps_log.tile([P, C], F32, tag="log", name="log")
            for hh in range(4):
                h = half * 4 + hh
                hp = h // 2
                kk = kTe[hp] if h % 2 == 0 else kTo[hp]
                nc.tensor.matmul(
                    ps_l[:, hh * P:(hh + 1) * P],
                    qT[hp][:, sl], kk[:, sl],
                    start=(hh == 0), stop=(hh == 3),
                )
            nc.scalar.activation(S[:, half * C:(half + 1) * C], ps_l[:],
                                 AF.Exp, scale=float(D) ** -0.5 / (WS * WS))
        Sm = p_att.tile([P, H * P], BF16, tag="att", name="att")
        S3 = S[:].rearrange("p (h s) -> p h s", h=H)
        Sm3 = Sm[:].rearrange("p (h s) -> p h s", h=H)
        m3 = mask[:].rearrange("p (o s) -> p o s", o=1)
        _, m_bc = broadcast_tensor_aps(S3, m3)
        nc.vector.tensor_tensor(out=Sm3, in0=S3, in1=m_bc, op=ALU.mult)
        rs = p_st.tile([P, H], F32, tag="rs", name="rs")
        nc.vector.tensor_reduce(
            out=rs[:], in_=Sm3,
            axis=mybir.AxisListType.X, op=ALU.add,
        )
        rr = p_st.tile([P, H], F32, tag="rr", name="rr")
        nc.vector.reciprocal(rr[:], rs[:])
        att = p_attT.tile([P, H * P], BF16, tag="attbuf", name="attbuf")
        att3 = att[:].rearrange("p (h s) -> p h s", h=H)
        r3 = rr[:].rearrange("p (h o) -> p h o", o=1)
        _, r_bc = broadcast_tensor_aps(Sm3, r3)
        nc.vector.tensor_tensor(out=att3, in0=Sm3, in1=r_bc, op=ALU.mult)
        attT = p_attT.tile([P, H * P], BF16, tag="attT", name="attT")
        nc.vector.transpose(attT[:], att[:])
        attTs[tj] = attT

    def emit_oT(tj):
        attT = attTs.pop(tj)
        vt = vs.pop(tj)
        ps_o = ps_mm.tile([P, C], F32, tag="mm", name="mm")
        for h in range(H):
            nc.tensor.matmul(
                ps_o[(h % 2) * 64:(h % 2) * 64 + 64,
                     (h // 2) * P:(h // 2) * P + P],
                vt[:, h * 64:(h + 1) * 64],
                attT[:, h * P:(h + 1) * P],
                start=True, stop=True,
            )
        ot = p_oT.tile([P, C], FP8, tag="oT", name="oT")
        oTs[tj] = ot
        nc.scalar.activation(ot[:], ps_o[:], AF.Identity, scale=WS)

    def emit_b4(tj):
        ot = oTs.pop(tj)
        xt = xs.pop(tj)
        ps = ps_mm.tile([P, C], F32, tag="mm", name="mm")
        oT4 = ot[:].rearrange("p (m c t) -> p m c t", m=2, c=2)
        for m in range(2):
            wps = wp[m][:].rearrange("p (c n) -> p c n", c=2)
            nc.tensor.matmul(
                ps[:], oT4[:, m], wps[:],
                start=(m == 0), stop=False, perf_mode=DR,
            )
        nc.tensor.matmul(ps[:], ones1[:], bpe[:], start=False, stop=True)
        yt = p_y.tile([P, C], F32, tag="y", name="y")
        ys[tj] = yt
        nc.vector.scalar_tensor_tensor(
            out=yt[:], in0=ps[:], scalar=1.0 / (WS * WS), in1=xt[:],
            op0=ALU.mult, op1=ALU.add,
        )

    def emit_b5(tj):
        g, i = divmod(tj, 4)
        if i == 0:
            h2Ts[g] = p_h2T.tile([P, KC * GT], FP8, tag="h2T", name="h2T")
        h2T3 = h2Ts[g][:].rearrange("p (c t) -> p c t", c=KC)
        ht2 = p_h.tile([P, C], BF16, tag="h", name="h")
        layernorm(ys[tj], ht2)
        pst = ps_t.tile([P, KC * P], BF16, tag="pst", name="pst")
        for c in range(KC):
            nc.tensor.transpose(pst[:, c * P:(c + 1) * P],
                                ht2[:, c * P:(c + 1) * P], ident[:])
        nc.vector.tensor_copy(h2T3[:, :, i * P:(i + 1) * P], pst[:])

    def make_c_chunks(g):
        """Deferred MLP of group g as emission closures."""
        h2Tg = h2Ts[g]
        r_t = []
        chunks = []
        h2T3 = h2Tg[:].rearrange("p (c t) -> p c t", c=KC)

        def c1(j):
            ps = ps_mm.tile([P, GT], F32, tag="mm", name="mm")
            for m in range(2):
                w1s = w1[m][:].rearrange("p (c j) -> p c j", c=2)
                nc.tensor.matmul(
                    ps[:], w1s[:, :, j * P:(j + 1) * P],
                    h2T3[:, 2 * m:2 * m + 2, :],
                    start=(m == 0), stop=(m == 1), perf_mode=DR,
                )
            if j % 2 == 0:
                r_t.append(p_r.tile([P, 2 * GT], FP8, tag="r", name="r"))
            rt = r_t[j // 2]
            nc.scalar.activation(rt[:, (j % 2) * GT:(j % 2 + 1) * GT], ps[:],
                                 AF.Relu, bias=b1t[:, j:j + 1])

        def c2t(i):
            tj = 4 * g + i
            ps = ps_mm.tile([P, C], F32, tag="mm", name="mm")
            for m in range(8):
                r3 = r_t[m][:].rearrange("p (c t) -> p c t", c=2)
                w2s = w2[m][:].rearrange("p (c n) -> p c n", c=2)
                nc.tensor.matmul(
                    ps[:], r3[:, :, i * P:(i + 1) * P], w2s[:],
                    start=(m == 0), stop=False, perf_mode=DR,
                )
            nc.tensor.matmul(ps[:], ones1[:], b2r[:], start=False, stop=True)
            yt = ys.pop(tj)
            ot = p_out.tile([P, C], F32, tag="out", name="out")
            nc.vector.scalar_tensor_tensor(
                out=ot[:], in0=ps[:], scalar=1.0 / (WS * WS), in1=yt[:],
                op0=ALU.mult, op1=ALU.add,
            )
            nc.sync.dma_start(out_d[tj * P:(tj + 1) * P, :], ot[:])

        from functools import partial
        for j in range(KM):
            chunks.append(partial(c1, j))
        for i in range(4):
            chunks.append(partial(c2t, i))
        return chunks

    # ---- prologue: group 0 input prep ----
    stage_a_dma(0)
    for i in range(4):
        stage_a_tile(0, i)

    # ---- flat cyclic pipeline over tiles ----
    for ti in range(ntiles + 7):
        g, i = divmod(ti, 4)
        if ti < ntiles:
            if i == 0:
                emit_b1(g)
                if g + 1 < ngroups:
                    stage_a_dma(g + 1)
            emit_v(ti)
            emit_att(ti)
        if 0 <= ti - 1 < ntiles:
            emit_oT(ti - 1)
        if 0 <= ti - 2 < ntiles:
            emit_b4(ti - 2)
        drain_c(DRAINS[i] if ti < ntiles else 1000)
        if 0 <= ti - 3 < ntiles:
            emit_b5(ti - 3)
            if (ti - 3) % 4 == 3:
                gc = (ti - 3) // 4
                pend_c.extend(make_c_chunks(gc))
                del h2Ts[gc]  # chunks hold their own reference
        if ti < ntiles and g + 1 < ngroups:
            stage_a_tile(g + 1, i)
        if i == 3 and g - 1 >= 0:
            qTs.pop(g - 1, None)
            hTs.pop(g - 1, None)

    drain_c(len(pend_c))


def _fp8_pair(w, ncols):
    """[K, ncols] f32 -> [K//256, 128, 2, ncols] fp8 paired chunks, flattened
    to [K//2, 2*ncols] for a contiguous per-tile DMA."""
    fp8 = ml_dtypes.float8_e4m3
    K = w.shape[0]
    npair = K // 256
    wp = w.reshape(npair, 2, P, ncols).transpose(0, 2, 1, 3)  # [m, p, c, cols]
    wp = np.clip(wp * WS, -240, 240)
    return np.ascontiguousarray(wp.reshape(npair * P, 2 * ncols)).astype(fp8)


def preprocess(inputs):
    """Host-side weight folding. Returns dict of extra device arrays."""
    f32 = np.float32
    bf16 = ml_dtypes.bfloat16
    fp8 = ml_dtypes.float8_e4m3
    g1 = np.asarray(inputs["ln1_g"], f32)
    b1n = np.asarray(inputs["ln1_b"], f32)
    g2 = np.asarray(inputs["ln2_g"], f32)
    b2n = np.asarray(inputs["ln2_b"], f32)
    wq = np.asarray(inputs["wq"], f32).transpose(1, 0, 2).reshape(C, HD)
    wk = np.asarray(inputs["wk"], f32).transpose(1, 0, 2).reshape(C, HD)
    wv = np.asarray(inputs["wv"], f32).transpose(1, 0, 2).reshape(C, HD)
    w_proj = np.asarray(inputs["w_proj"], f32)
    b_proj = np.asarray(inputs["b_proj"], f32)
    w1 = np.asarray(inputs["w1"], f32)
    b1 = np.asarray(inputs["b1"], f32)
    w2 = np.asarray(inputs["w2"], f32)
    b2 = np.asarray(inputs["b2"], f32)

    wqg = wq * g1[:, None]
    wkg = wk * g1[:, None]
    wvg = wv * g1[:, None]
    w1g = w1 * g2[:, None]

    wqk = np.concatenate([wqg, wkg], axis=1)          # [C, 1024]
    bqk = b1n @ wqk                                   # [1024]
    bias_v = b1n @ wvg                                # [512]
    bpe = (bias_v @ w_proj + b_proj)[None, :]         # [1, 512]
    bias1 = b2n @ w1g + b1                            # [2048]

    # w_proj packed for fp8 DR with oT's (d-fold, head-pair) layout:
    # lhsT (p, c, t) = oT[p, (2m+c)*128+t] = WS*o[t, hd(p, 2m+c)] with
    # hd(p, q) = (2q + p//64)*64 + p%64, so rhs (p, c, n) must be
    # WS*w_proj[hd(p, 2m+c), n].
    wp4 = w_proj.reshape(H, D, C)
    wpp = np.zeros((2, P, 2, C), f32)
    for m in range(2):
        for e in range(2):
            for c in range(2):
                wpp[m, 64 * e:64 * (e + 1), c, :] = wp4[2 * (2 * m + c) + e]
    wpp = np.clip(wpp * WS, -240, 240).reshape(2 * P, 2 * C)

    # multiplicative mask: 1 on allowed (block-diag causal), 0 elsewhere
    mask = np.zeros((P, P), f32)
    tril = np.tril(np.ones((T, T), bool))
    for b in range(P // T):
        blk = mask[b * T:(b + 1) * T, b * T:(b + 1) * T]
        blk[tril] = 1.0

    # bqk laid out [128, 12]: cols 0-3 q biases (full), 4-7 k biases with
    # odd-head rows zeroed, 8-11 k biases with even-head rows zeroed; plus
    # the matching 0/1 row-scale masks in scm.
    bqkw = bqk * WS
    bqkx = np.zeros((P, 12), f32)
    for j in range(4):
        bqkx[:, j] = bqkw[j * P:(j + 1) * P]
        kcol = bqkw[HD + j * P:HD + (j + 1) * P]
        bqkx[:64, 4 + j] = kcol[:64]
        bqkx[64:, 8 + j] = kcol[64:]
    scm = np.zeros((P, 2), f32)
    scm[:64, 0] = 1.0
    scm[64:, 1] = 1.0

    return {
        "wqk": _fp8_pair(wqk, 2 * HD),
        "wv": _fp8_pair(wvg, HD),
        "wp": wpp.astype(fp8),
        "w1": _fp8_pair(w1g, M1),
        "w2": _fp8_pair(w2, C),
        "bqk": bqkx,
        "scm": scm,
        "b1": (bias1 * WS).astype(f32),
        "b2r": (b2 * WS * WS)[None, :].astype(bf16),
        "bpe": (bpe * WS * WS).astype(bf16),
        "mask": mask.astype(bf16),
        "ident": np.eye(P, dtype=bf16),
    }


def _patch_act_tables():
    """Make every activation func we use resolve to the single table set
    `natural_log_exp_and_others` (it contains Ln, Exp, Identity and Relu),
    so bacc's table-load pass emits one load instead of thrashing between
    `natural_log` and `exp_and_others` (~2.7us per switch)."""
    import concourse.bacc as _bacc_mod
    import concourse.hw_specs as _hw
    if getattr(_bacc_mod, "_ant_act_tables_patched", False):
        return
    _orig = _hw.get_activation_tables
    ours = {AF.Ln, AF.Exp, AF.Identity, AF.Relu, AF.Copy}

    def patched(arch):
        tables = _orig(arch)
        out = {}
        for name, funcs in tables.items():
            if name == "natural_log_exp_and_others":
                out[name] = funcs
            else:
                out[name] = funcs - ours
        return out

    _bacc_mod.get_activation_tables = patched
    _bacc_mod._ant_act_tables_patched = True


def build(ntok=NTOK_FULL):
    """Build the Bass program; returns nc."""
    from contextlib import ExitStack

    _patch_act_tables()
    nc = bacc.Bacc("TRN2", target_bir_lowering=False, debug=False,
                   enable_asserts=False, num_devices=NCORES)
    ins = {
        "x": nc.dram_tensor("x", [ntok, C], F32, kind="ExternalInput").ap(),
        "wqk": nc.dram_tensor("wqk", [2 * P, 2 * 2 * HD], FP8,
                              kind="ExternalInput").ap(),
        "wv": nc.dram_tensor("wv", [2 * P, 2 * HD], FP8,
                             kind="ExternalInput").ap(),
        "wp": nc.dram_tensor("wp", [2 * P, 2 * C], FP8,
                             kind="ExternalInput").ap(),
        "w1": nc.dram_tensor("w1", [2 * P, 2 * M1], FP8,
                             kind="ExternalInput").ap(),
        "w2": nc.dram_tensor("w2", [8 * P, 2 * C], FP8,
                             kind="ExternalInput").ap(),
        "bqk": nc.dram_tensor("bqk", [P, 12], F32, kind="ExternalInput").ap(),
        "scm": nc.dram_tensor("scm", [P, 2], F32, kind="ExternalInput").ap(),
        "b1": nc.dram_tensor("b1", [M1], F32, kind="ExternalInput").ap(),
        "b2r": nc.dram_tensor("b2r", [1, C], BF16, kind="ExternalInput").ap(),
        "bpe": nc.dram_tensor("bpe", [1, C], BF16, kind="ExternalInput").ap(),
        "mask": nc.dram_tensor("mask", [P, P], BF16, kind="ExternalInput").ap(),
        "ident": nc.dram_tensor("ident", [P, P], BF16, kind="ExternalInput").ap(),
    }
    outs = {
        "out": nc.dram_tensor("out", [ntok, C], F32, kind="ExternalOutput").ap(),
    }
    with ExitStack() as ctx:
        tc = ctx.enter_context(tile.TileContext(nc))
        emit_block(ctx, tc, outs, ins, ntok)
    nc.finalize()
    return nc


def kernel(**inputs):
    x = np.ascontiguousarray(np.asarray(inputs["x"], np.float32))
    consts = preprocess(inputs)
    nc = build(NTOK_FULL)
    xs = x.reshape(NCORES, NTOK_FULL, C)
    in_maps = [dict(consts, x=np.ascontiguousarray(xs[c])) for c in range(NCORES)]
    res = run_bass_kernel_spmd(nc, in_maps, core_ids=list(range(NCORES)))
    out = np.stack([res.results[c]["out"] for c in range(NCORES)], axis=0)
    return out.reshape(B, T, C).astype(np.float32)


if __name__ == "__main__":
    rng = np.random.default_rng(0)
    fake = {
        "x": rng.standard_normal((B, T, C), dtype=np.float32),
        "ln1_g": np.ones(C, np.float32), "ln1_b": np.zeros(C, np.float32),
        "wq": rng.standard_normal((H, C, D), dtype=np.float32) * 0.02,
        "wk": rng.standard_normal((H, C, D), dtype=np.float32) * 0.02,
        "wv": rng.standard_normal((H, C, D), dtype=np.float32) * 0.02,
        "w_proj": rng.standard_normal((HD, C), dtype=np.float32) * 0.02,
        "b_proj": np.zeros(C, np.float32),
        "ln2_g": np.ones(C, np.float32), "ln2_b": np.zeros(C, np.float32),
        "w1": rng.standard_normal((C, M1), dtype=np.float32) * 0.02,
        "b1": np.zeros(M1, np.float32),
        "w2": rng.standard_normal((M1, C), dtype=np.float32) * 0.02,
        "b2": np.zeros(C, np.float32),
    }
    out = kernel(**fake)
    print("kernel ran, out shape", out.shape)


# revision 14
# speedup vs baseline: 1.1056x; 1.1056x over previous
"""Trainium2 Bass/Tile kernel for a dense transformer block.

Math (per batch element b, T=16 tokens, C=512, H=8 heads, D=64):
    h  = LN(x; ln1_g, ln1_b)
    q,k,v = per-head projections of h
    att = causal-softmax(q k^T / sqrt(D)); o = att v (heads concatenated)
    y  = o @ w_proj + b_proj + x
    f  = relu(LN(y; ln2_g, ln2_b) @ w1 + b1) @ w2 + b2
    out = f + y

Distribution: pure data parallel over the batch dim (4096) across 8
NeuronCores; weights replicated; no collectives.

v4: fully cyclic software pipeline over 128-token tiles; every
iteration emits the same work mix (v+logits+softmax for tile ti,
o-matmuls for ti-1, proj+LN2-stats for ti-2, LN2-apply for ti-3, LN1
stats/apply for tiles 4-5 ahead, plus a steady drip of deferred MLP
chunks) so the PE never idles long enough for the HAM clock gate to
re-throttle.
  * causal mask stays additive on the PE (accumulating ident x mask
    matmul into the logits PSUM) -- off the DVE softmax chain.
  * the two exp halves are one [128,1024] activation over a 2-bank
    PSUM tile.
  * att row-scale is one stride-0-broadcast tensor_tensor.
  * LN stats (bn_stats/aggr + the two tiny rstd activations) run 1-5
    iterations ahead of their apply, so the tiny scalar ops never
    block urgent scalar work (exp / evictions) in the FIFO.
  * c2 (MLP down-proj) emits token-major (lhsT = r slices, rhs = w2
    row-paired fp8 DR): no fT transposes; eviction fused with +y.
  * proj runs fp8 DoubleRow (oT evicted WS-scaled fp8); q,k fp8.
  * rank-1 bias matmuls (bpe, b2) are skipped at build time when the
    folded biases are exactly zero (true for this model's inputs).
"""

import sys

sys.path.insert(0, "/opt/trn_rl_repo")

import numpy as np
import ml_dtypes

import concourse.bass as bass
import concourse.tile as tile
from concourse import bacc, mybir
from concourse.bass import broadcast_tensor_aps
from concourse.bass_utils import run_bass_kernel_spmd

F32 = mybir.dt.float32
BF16 = mybir.dt.bfloat16
FP8 = mybir.dt.float8e4
DR = mybir.MatmulPerfMode.DoubleRow
AF = mybir.ActivationFunctionType
ALU = mybir.AluOpType

WS = 32.0

NCORES = 8
B, T, C, H, D = 4096, 16, 512, 8, 64
HD = H * D          # 512
M1 = 4 * C          # 2048
EPS = 1e-5
BL = B // NCORES    # 512 batch elems per core
NTOK_FULL = BL * T  # 8192 tokens per core
P = 128             # partitions
GT = 512            # tokens per group
KC = C // P         # 4 c-chunks
KM = M1 // P        # 16 hidden chunks

# deferred-MLP drain counts by tile phase (chunks for group g are created
# at ti=4g+6; phase 0 is light because B1 runs there)
DRAINS = (2, 6, 6, 6)

# set by preprocess(): which folded biases are exactly zero (lets build()
# skip the rank-1 bias matmuls; correctness-neutral, they are just zeros)
_ZFLAGS = {"bpe": False, "b2": False}


def emit_block(ctx, tc, outs, ins, ntok, zflags):
    """Emit the transformer-block program. outs/ins: dicts of DRAM APs."""
    nc = tc.nc
    x_d = ins["x"]
    wqk_d = ins["wqk"]
    wv_d = ins["wv"]
    wp_d = ins["wp"]
    w1_d = ins["w1"]
    w2_d = ins["w2"]
    out_d = outs["out"]

    ngroups = ntok // GT
    ntiles = ntok // P
    assert ntok % GT == 0

    consts = ctx.enter_context(tc.tile_pool(name="consts", bufs=1))

    wqk = [consts.tile([P, 2 * 2 * HD], FP8, tag=f"wqk{m}", name=f"wqk{m}")
           for m in range(2)]
    wv = [consts.tile([P, 2 * HD], FP8, tag=f"wv{m}", name=f"wv{m}")
          for m in range(2)]
    wp = [consts.tile([P, 2 * C], FP8, tag=f"wp{m}", name=f"wp{m}")
          for m in range(2)]
    w1 = [consts.tile([P, 2 * M1], FP8, tag=f"w1{m}", name=f"w1{m}")
          for m in range(2)]
    w2 = [consts.tile([P, 2 * C], FP8, tag=f"w2{m}", name=f"w2{m}")
          for m in range(8)]

    bqk = consts.tile([P, 12], F32, tag="bqk", name="bqk")
    scm = consts.tile([P, 2], F32, tag="scm", name="scm")
    b1t = consts.tile([P, KM], F32, tag="b1t", name="b1t")
    nc.sync.dma_start(bqk[:], ins["bqk"][:, :])
    nc.sync.dma_start(scm[:], ins["scm"][:, :])
    nc.sync.dma_start(b1t[:], ins["b1"].rearrange("(j p) -> p j", p=P))

    if not zflags["bpe"]:
        bpe = consts.tile([1, C], BF16, tag="bpe", name="bpe")
        nc.sync.dma_start(bpe[:], ins["bpe"][:, :])
    if not zflags["b2"]:
        b2r = consts.tile([1, C], BF16, tag="b2r", name="b2r")
        nc.sync.dma_start(b2r[:], ins["b2r"][:, :])
    # additive causal mask, 0 on allowed block-diag positions and
    # -300*WS^2 elsewhere (the exp scale is 0.125/WS^2)
    mask = consts.tile([P, KC * P], BF16, tag="mask", name="mask")
    nc.sync.dma_start(mask[:], ins["mask"][:, :])
    ident = consts.tile([P, P], BF16, tag="ident", name="ident")
    nc.sync.dma_start(ident[:], ins["ident"][:, :])
    ones1 = consts.tile([1, P], BF16, tag="ones1", name="ones1")
    nc.vector.memset(ones1[:], 1.0)
    epst = consts.tile([P, 1], F32, tag="epst", name="epst")
    nc.vector.memset(epst[:], EPS)

    def dma_weights_front():
        """Weights needed first (B1 / v of tile 0)."""
        for m in range(2):
            nc.sync.dma_start(wqk[m][:], wqk_d[m * P:(m + 1) * P, :])
            nc.sync.dma_start(wv[m][:], wv_d[m * P:(m + 1) * P, :])

    def dma_weights_rest():
        """Weights first needed several iterations in (b4 / MLP chunks)."""
        for m in range(2):
            nc.sync.dma_start(wp[m][:], wp_d[m * P:(m + 1) * P, :])
            nc.sync.dma_start(w1[m][:], w1_d[m * P:(m + 1) * P, :])
        for m in range(8):
            nc.sync.dma_start(w2[m][:], w2_d[m * P:(m + 1) * P, :])

    # --- working pools ---
    p_x = ctx.enter_context(tc.tile_pool(name="p_x", bufs=14))
    p_h = ctx.enter_context(tc.tile_pool(name="p_h", bufs=3))
    p_hT = ctx.enter_context(tc.tile_pool(name="p_hT", bufs=2))
    p_qk = ctx.enter_context(tc.tile_pool(name="p_qk", bufs=24))
    p_v = ctx.enter_context(tc.tile_pool(name="p_v", bufs=6))
    p_S = ctx.enter_context(tc.tile_pool(name="p_S", bufs=4))
    p_attT = ctx.enter_context(tc.tile_pool(name="p_attT", bufs=4))
    p_oT = ctx.enter_context(tc.tile_pool(name="p_oT", bufs=6))
    p_y = ctx.enter_context(tc.tile_pool(name="p_y", bufs=10))
    p_h2T = ctx.enter_context(tc.tile_pool(name="p_h2T", bufs=3))
    p_r = ctx.enter_context(tc.tile_pool(name="p_r", bufs=20))
    p_out = ctx.enter_context(tc.tile_pool(name="p_out", bufs=4))
    p_st = ctx.enter_context(tc.tile_pool(name="p_st", bufs=12))

    ps_mm = ctx.enter_context(tc.tile_pool(name="ps_mm", bufs=4, space="PSUM"))
    ps_log = ctx.enter_context(tc.tile_pool(name="ps_log", bufs=1, space="PSUM"))
    ps_t = ctx.enter_context(tc.tile_pool(name="ps_t", bufs=2, space="PSUM"))

    # --- cross-iteration state ---
    xs = {}        # tj -> x tile
    hTs = {}       # g -> hT group tile
    qTs = {}       # g -> (qT, kTe, kTo) lists
    vs = {}        # tj -> v tile
    oTs = {}       # tj -> oT tile
    attTs = {}     # tj -> attT tile
    ys = {}        # tj -> y tile
    h2Ts = {}      # g -> h2T group tile
    ln1s = {}      # tj -> (mv, rstd)
    ln2s = {}      # tj -> (mv, rstd)
    pend_c = []

    def drain_c(n):
        for _ in range(min(n, len(pend_c))):
            pend_c.pop(0)()

    def ln_stats(src, store, tj):
        st = p_st.tile([P, 6], F32, tag="bn", name="bn")
        mv = p_st.tile([P, 2], F32, tag="mv", name="mv")
        nc.vector.bn_stats(st[:], src[:])
        nc.vector.bn_aggr(mv[:], st[:])
        lnv = p_st.tile([P, 1], F32, tag="lnv", name="lnv")
        rstd = p_st.tile([P, 1], F32, tag="rstd", name="rstd")
        nc.scalar.activation(lnv[:], mv[:, 1:2], AF.Ln, bias=epst[:])
        nc.scalar.activation(rstd[:], lnv[:], AF.Exp, scale=-0.5)
        store[tj] = (mv, rstd)

    def ln_apply(src, store, tj, h_t):
        mv, rstd = store.pop(tj)
        nc.vector.tensor_scalar(
            out=h_t[:], in0=src[:],
            scalar1=mv[:, 0:1], scalar2=rstd[:],
            op0=ALU.subtract, op1=ALU.mult,
        )

    def stage_a_dma(g):
        for i in range(4):
            tj = 4 * g + i
            xt = p_x.tile([P, C], F32, tag="x", name="x")
            xs[tj] = xt
            nc.sync.dma_start(xt[:], x_d[tj * P:(tj + 1) * P, :])

    def ln1_stats(tj):
        ln_stats(xs[tj], ln1s, tj)

    def ln1_apply(tj):
        """LN1 apply + feature-major transpose of tile tj into hTs[g]."""
        g, i = divmod(tj, 4)
        if i == 0:
            hTs[g] = p_hT.tile([P, KC * GT], FP8, tag="hT", name="hT")
        hT3 = hTs[g][:].rearrange("p (c t) -> p c t", c=KC)
        ht = p_h.tile([P, C], BF16, tag="h", name="h")
        ln_apply(xs[tj], ln1s, tj, ht)
        pst = ps_t.tile([P, KC * P], BF16, tag="pst", name="pst")
        for c in range(KC):
            nc.tensor.transpose(pst[:, c * P:(c + 1) * P],
                                ht[:, c * P:(c + 1) * P], ident[:])
        nc.vector.tensor_copy(hT3[:, :, i * P:(i + 1) * P], pst[:])

    def emit_b1(g):
        """q^T, k^T for group g. k is evicted twice per chunk with 0/1
        per-partition scale masks (zero-padded single-head copies)."""
        hT3 = hTs[g][:].rearrange("p (c t) -> p c t", c=KC)
        qT = [p_qk.tile([P, GT], FP8, tag="qk", name="qk") for _ in range(KC)]
        kTe = [p_qk.tile([P, GT], FP8, tag="qk", name="qk") for _ in range(KC)]
        kTo = [p_qk.tile([P, GT], FP8, tag="qk", name="qk") for _ in range(KC)]
        qTs[g] = (qT, kTe, kTo)
        for j in range(8):
            ps = ps_mm.tile([P, GT], F32, tag="mm", name="mm")
            for m in range(2):
                wqks = wqk[m][:].rearrange("p (c j) -> p c j", c=2)
                nc.tensor.matmul(
                    ps[:], wqks[:, :, j * P:(j + 1) * P],
                    hT3[:, 2 * m:2 * m + 2, :],
                    start=(m == 0), stop=(m == 1), perf_mode=DR,
                )
            if j < 4:
                nc.scalar.activation(qT[j][:], ps[:], AF.Identity,
                                     bias=bqk[:, j:j + 1])
            else:
                hp = j - 4
                nc.scalar.activation(kTe[hp][:], ps[:], AF.Identity,
                                     scale=scm[:, 0:1], bias=bqk[:, 4 + hp:5 + hp])
                nc.scalar.activation(kTo[hp][:], ps[:], AF.Identity,
                                     scale=scm[:, 1:2], bias=bqk[:, 8 + hp:9 + hp])

    def emit_v(tj):
        g, i = divmod(tj, 4)
        hT3 = hTs[g][:].rearrange("p (c t) -> p c t", c=KC)
        ps = ps_mm.tile([P, HD], F32, tag="mm", name="mm")
        for m in range(2):
            wvs = wv[m][:].rearrange("p (c d) -> p c d", c=2)
            nc.tensor.matmul(
                ps[:], hT3[:, 2 * m:2 * m + 2, i * P:(i + 1) * P],
                wvs[:],
                start=(m == 0), stop=(m == 1), perf_mode=DR,
            )
        vt = p_v.tile([P, HD], BF16, tag="v", name="v")
        vs[tj] = vt
        nc.scalar.activation(vt[:], ps[:], AF.Identity, scale=1.0 / WS)

    def emit_att(tj):
        """logits + additive mask (PE) + one merged exp + normalize + attT."""
        g, i = divmod(tj, 4)
        qT, kTe, kTo = qTs[g]
        sl = slice(i * P, (i + 1) * P)
        ps_l = ps_log.tile([P, 2 * C], F32, tag="log", name="log")
        for half in range(2):
            hof = half * C
            for hh in range(4):
                h = half * 4 + hh
                hp = h // 2
                kk = kTe[hp] if h % 2 == 0 else kTo[hp]
                nc.tensor.matmul(
                    ps_l[:, hof + hh * P:hof + (hh + 1) * P],
                    qT[hp][:, sl], kk[:, sl],
                    start=(hh == 0), stop=False,
                )
            nc.tensor.matmul(ps_l[:, hof:hof + C], ident[:], mask[:],
                             start=False, stop=True)
        S = p_S.tile([P, H * P], FP8, tag="S", name="S")
        nc.scalar.activation(S[:], ps_l[:], AF.Exp,
                             scale=float(D) ** -0.5 / (WS * WS))
        S3 = S[:].rearrange("p (h s) -> p h s", h=H)
        rs = p_st.tile([P, H], F32, tag="rs", name="rs")
        nc.vector.tensor_reduce(
            out=rs[:], in_=S3,
            axis=mybir.AxisListType.X, op=ALU.add,
        )
        rr = p_st.tile([P, H], F32, tag="rr", name="rr")
        nc.vector.reciprocal(rr[:], rs[:])
        att = p_attT.tile([P, H * P], FP8, tag="attbuf", name="attbuf")
        att3 = att[:].rearrange("p (h s) -> p h s", h=H)
        r3 = rr[:].rearrange("p (h o) -> p h o", o=1)
        _, r_bc = broadcast_tensor_aps(S3, r3)
        nc.vector.tensor_tensor(out=att3, in0=S3, in1=r_bc, op=ALU.mult)
        attT = p_attT.tile([P, H * P], FP8, tag="attT", name="attT")
        nc.vector.transpose(attT[:], att[:])
        attTs[tj] = attT

    def emit_oT(tj):
        attT = attTs.pop(tj)
        vt = vs.pop(tj)
        ps_o = ps_mm.tile([P, C], F32, tag="mm", name="mm")
        for h in range(H):
            nc.tensor.matmul(
                ps_o[(h % 2) * 64:(h % 2) * 64 + 64,
                     (h // 2) * P:(h // 2) * P + P],
                vt[:, h * 64:(h + 1) * 64],
                attT[:, h * P:(h + 1) * P],
                start=True, stop=True,
            )
        ot = p_oT.tile([P, C], FP8, tag="oT", name="oT")
        oTs[tj] = ot
        nc.scalar.activation(ot[:], ps_o[:], AF.Identity, scale=WS)

    def emit_b4(tj):
        ot = oTs.pop(tj)
        xt = xs.pop(tj)
        ps = ps_mm.tile([P, C], F32, tag="mm", name="mm")
        oT4 = ot[:].rearrange("p (m c t) -> p m c t", m=2, c=2)
        for m in range(2):
            wps = wp[m][:].rearrange("p (c n) -> p c n", c=2)
            nc.tensor.matmul(
                ps[:], oT4[:, m], wps[:],
                start=(m == 0), stop=(m == 1 and zflags["bpe"]), perf_mode=DR,
            )
        if not zflags["bpe"]:
            nc.tensor.matmul(ps[:], ones1[:], bpe[:], start=False, stop=True)
        yt = p_y.tile([P, C], F32, tag="y", name="y")
        ys[tj] = yt
        nc.vector.scalar_tensor_tensor(
            out=yt[:], in0=ps[:], scalar=1.0 / (WS * WS), in1=xt[:],
            op0=ALU.mult, op1=ALU.add,
        )
        # LN2 stats run here, one iteration ahead of the apply, so the
        # tiny rstd activations never block urgent scalar work.
        ln_stats(yt, ln2s, tj)

    def emit_b5(tj):
        g, i = divmod(tj, 4)
        if i == 0:
            h2Ts[g] = p_h2T.tile([P, KC * GT], FP8, tag="h2T", name="h2T")
        h2T3 = h2Ts[g][:].rearrange("p (c t) -> p c t", c=KC)
        ht2 = p_h.tile([P, C], BF16, tag="h", name="h")
        ln_apply(ys[tj], ln2s, tj, ht2)
        pst = ps_t.tile([P, KC * P], BF16, tag="pst", name="pst")
        for c in range(KC):
            nc.tensor.transpose(pst[:, c * P:(c + 1) * P],
                                ht2[:, c * P:(c + 1) * P], ident[:])
        nc.vector.tensor_copy(h2T3[:, :, i * P:(i + 1) * P], pst[:])

    def make_c_chunks(g):
        """Deferred MLP of group g as emission closures."""
        h2Tg = h2Ts[g]
        r_t = []
        chunks = []
        h2T3 = h2Tg[:].rearrange("p (c t) -> p c t", c=KC)

        def c1(j):
            ps = ps_mm.tile([P, GT], F32, tag="mm", name="mm")
            for m in range(2):
                w1s = w1[m][:].rearrange("p (c j) -> p c j", c=2)
                nc.tensor.matmul(
                    ps[:], w1s[:, :, j * P:(j + 1) * P],
                    h2T3[:, 2 * m:2 * m + 2, :],
                    start=(m == 0), stop=(m == 1), perf_mode=DR,
                )
            if j % 2 == 0:
                r_t.append(p_r.tile([P, 2 * GT], FP8, tag="r", name="r"))
            rt = r_t[j // 2]
            nc.scalar.activation(rt[:, (j % 2) * GT:(j % 2 + 1) * GT], ps[:],
                                 AF.Relu, bias=b1t[:, j:j + 1])

        def c2t(i):
            tj = 4 * g + i
            ps = ps_mm.tile([P, C], F32, tag="mm", name="mm")
            for m in range(8):
                r3 = r_t[m][:].rearrange("p (c t) -> p c t", c=2)
                w2s = w2[m][:].rearrange("p (c n) -> p c n", c=2)
                nc.tensor.matmul(
                    ps[:], r3[:, :, i * P:(i + 1) * P], w2s[:],
                    start=(m == 0), stop=(m == 7 and zflags["b2"]),
                    perf_mode=DR,
                )
            if not zflags["b2"]:
                nc.tensor.matmul(ps[:], ones1[:], b2r[:], start=False,
                                 stop=True)
            yt = ys.pop(tj)
            ot = p_out.tile([P, C], F32, tag="out", name="out")
            nc.vector.scalar_tensor_tensor(
                out=ot[:], in0=ps[:], scalar=1.0 / (WS * WS), in1=yt[:],
                op0=ALU.mult, op1=ALU.add,
            )
            nc.sync.dma_start(out_d[tj * P:(tj + 1) * P, :], ot[:])

        from functools import partial
        for j in range(KM):
            chunks.append(partial(c1, j))
        for i in range(4):
            chunks.append(partial(c2t, i))
        return chunks

    # ---- prologue: group 0 (and group 1 stats head-start) ----
    # DMA order: x tiles + front weights first so compute starts early;
    # the bulk weights (wp/w1/w2, ~2MB) follow and overlap the first
    # iterations (they are not needed until b4 / the MLP chunks).
    stage_a_dma(0)
    stage_a_dma(1)
    dma_weights_front()
    for tj in range(5):
        ln1_stats(tj)
    for tj in range(4):
        ln1_apply(tj)
    dma_weights_rest()

    # ---- flat cyclic pipeline over tiles ----
    for ti in range(ntiles + 7):
        g, i = divmod(ti, 4)
        # LN2-apply + transposes first: gives the PE immediate work each
        # iteration and keeps the apply off the back of the softmax chain
        # in the DVE FIFO.
        if 0 <= ti - 3 < ntiles:
            emit_b5(ti - 3)
            if (ti - 3) % 4 == 3:
                gc = (ti - 3) // 4
                pend_c.extend(make_c_chunks(gc))
                del h2Ts[gc]
        if ti < ntiles:
            if i == 0:
                emit_b1(g)
            emit_v(ti)
            emit_att(ti)
        if 0 <= ti - 1 < ntiles:
            emit_oT(ti - 1)
        if 0 <= ti - 2 < ntiles:
            emit_b4(ti - 2)
        drain_c(DRAINS[i] if ti < ntiles else 1000)
        if ti < ntiles:
            if i == 2 and g + 2 < ngroups:
                stage_a_dma(g + 2)
            if ti + 5 < ntiles:
                ln1_stats(ti + 5)
            if ti + 4 < ntiles:
                ln1_apply(ti + 4)
        if i == 3 and g - 1 >= 0:
            qTs.pop(g - 1, None)
            hTs.pop(g - 1, None)

    drain_c(len(pend_c))


def _fp8_pair(w, ncols):
    """[K, ncols] f32 -> [K//256, 128, 2, ncols] fp8 paired chunks, flattened
    to [K//2, 2*ncols] for a contiguous per-tile DMA."""
    fp8 = ml_dtypes.float8_e4m3
    K = w.shape[0]
    npair = K // 256
    wp = w.reshape(npair, 2, P, ncols).transpose(0, 2, 1, 3)  # [m, p, c, cols]
    wp = np.clip(wp * WS, -240, 240)
    return np.ascontiguousarray(wp.reshape(npair * P, 2 * ncols)).astype(fp8)


def preprocess(inputs):
    """Host-side weight folding. Returns dict of extra device arrays."""
    f32 = np.float32
    bf16 = ml_dtypes.bfloat16
    fp8 = ml_dtypes.float8_e4m3
    g1 = np.asarray(inputs["ln1_g"], f32)
    b1n = np.asarray(inputs["ln1_b"], f32)
    g2 = np.asarray(inputs["ln2_g"], f32)
    b2n = np.asarray(inputs["ln2_b"], f32)
    wq = np.asarray(inputs["wq"], f32).transpose(1, 0, 2).reshape(C, HD)
    wk = np.asarray(inputs["wk"], f32).transpose(1, 0, 2).reshape(C, HD)
    wv = np.asarray(inputs["wv"], f32).transpose(1, 0, 2).reshape(C, HD)
    w_proj = np.asarray(inputs["w_proj"], f32)
    b_proj = np.asarray(inputs["b_proj"], f32)
    w1 = np.asarray(inputs["w1"], f32)
    b1 = np.asarray(inputs["b1"], f32)
    w2 = np.asarray(inputs["w2"], f32)
    b2 = np.asarray(inputs["b2"], f32)

    wqg = wq * g1[:, None]
    wkg = wk * g1[:, None]
    wvg = wv * g1[:, None]
    w1g = w1 * g2[:, None]

    wqk = np.concatenate([wqg, wkg], axis=1)          # [C, 1024]
    bqk = b1n @ wqk                                   # [1024]
    bias_v = b1n @ wvg                                # [512]
    bpe = (bias_v @ w_proj + b_proj)[None, :]         # [1, 512]
    bias1 = b2n @ w1g + b1                            # [2048]

    _ZFLAGS["bpe"] = bool(np.all(bpe == 0.0))
    _ZFLAGS["b2"] = bool(np.all(b2 == 0.0))

    # w_proj packed for fp8 DR with oT's (d-fold, head-pair) layout.
    wp4 = w_proj.reshape(H, D, C)
    wpp = np.zeros((2, P, 2, C), f32)
    for m in range(2):
        for e in range(2):
            for c in range(2):
                wpp[m, 64 * e:64 * (e + 1), c, :] = wp4[2 * (2 * m + c) + e]
    wpp = np.clip(wpp * WS, -240, 240).reshape(2 * P, 2 * C)

    # additive mask: 0 on allowed (block-diag causal), -300*WS^2 elsewhere
    mask = np.full((P, P), -300.0 * WS * WS, f32)
    tril = np.tril(np.ones((T, T), bool))
    for b in range(P // T):
        blk = mask[b * T:(b + 1) * T, b * T:(b + 1) * T]
        blk[tril] = 0.0

    # bqk [128, 12]: cols 0-3 q biases, 4-7 k biases (odd rows zeroed),
    # 8-11 k biases (even rows zeroed); scm holds the 0/1 row masks.
    bqkw = bqk * WS
    bqkx = np.zeros((P, 12), f32)
    for j in range(4):
        bqkx[:, j] = bqkw[j * P:(j + 1) * P]
        kcol = bqkw[HD + j * P:HD + (j + 1) * P]
        bqkx[:64, 4 + j] = kcol[:64]
        bqkx[64:, 8 + j] = kcol[64:]
    scm = np.zeros((P, 2), f32)
    scm[:64, 0] = 1.0
    scm[64:, 1] = 1.0

    return {
        "wqk": _fp8_pair(wqk, 2 * HD),
        "wv": _fp8_pair(wvg, HD),
        "wp": wpp.astype(fp8),
        "w1": _fp8_pair(w1g, M1),
        "w2": _fp8_pair(w2, C),
        "bqk": bqkx,
        "scm": scm,
        "b1": (bias1 * WS).astype(f32),
        "b2r": (b2 * WS * WS)[None, :].astype(bf16),
        "bpe": (bpe * WS * WS).astype(bf16),
        "mask": np.tile(mask, (1, KC)).astype(bf16),
        "ident": np.eye(P, dtype=bf16),
    }


def _patch_act_tables():
    """Make every activation func we use resolve to the single table set
    `natural_log_exp_and_others` (it contains Ln, Exp, Identity and Relu),
    so bacc's table-load pass emits one load instead of thrashing between
    `natural_log` and `exp_and_others` (~2.7us per switch)."""
    import concourse.bacc as _bacc_mod
    import concourse.hw_specs as _hw
    if getattr(_bacc_mod, "_ant_act_tables_patched", False):
        return
    _orig = _hw.get_activation_tables
    ours = {AF.Ln, AF.Exp, AF.Identity, AF.Relu, AF.Copy}

    def patched(arch):
        tables = _orig(arch)
        out = {}
        for name, funcs in tables.items():
            if name == "natural_log_exp_and_others":
                out[name] = funcs
            else:
                out[name] = funcs - ours
        return out

    _bacc_mod.get_activation_tables = patched
    _bacc_mod._ant_act_tables_patched = True


def build(ntok=NTOK_FULL, zflags=None):
    """Build the Bass program; returns nc."""
    from contextlib import ExitStack

    if zflags is None:
        zflags = dict(_ZFLAGS)
    _patch_act_tables()
    nc = bacc.Bacc("TRN2", target_bir_lowering=False, debug=False,
                   enable_asserts=False, num_devices=NCORES)
    ins = {
        "x": nc.dram_tensor("x", [ntok, C], F32, kind="ExternalInput").ap(),
        "wqk": nc.dram_tensor("wqk", [2 * P, 2 * 2 * HD], FP8,
                              kind="ExternalInput").ap(),
        "wv": nc.dram_tensor("wv", [2 * P, 2 * HD], FP8,
                             kind="ExternalInput").ap(),
        "wp": nc.dram_tensor("wp", [2 * P, 2 * C], FP8,
                             kind="ExternalInput").ap(),
        "w1": nc.dram_tensor("w1", [2 * P, 2 * M1], FP8,
                             kind="ExternalInput").ap(),
        "w2": nc.dram_tensor("w2", [8 * P, 2 * C], FP8,
                             kind="ExternalInput").ap(),
        "bqk": nc.dram_tensor("bqk", [P, 12], F32, kind="ExternalInput").ap(),
        "scm": nc.dram_tensor("scm", [P, 2], F32, kind="ExternalInput").ap(),
        "b1": nc.dram_tensor("b1", [M1], F32, kind="ExternalInput").ap(),
        "b2r": nc.dram_tensor("b2r", [1, C], BF16, kind="ExternalInput").ap(),
        "bpe": nc.dram_tensor("bpe", [1, C], BF16, kind="ExternalInput").ap(),
        "mask": nc.dram_tensor("mask", [P, KC * P], BF16,
                               kind="ExternalInput").ap(),
        "ident": nc.dram_tensor("ident", [P, P], BF16,
                                kind="ExternalInput").ap(),
    }
    outs = {
        "out": nc.dram_tensor("out", [ntok, C], F32, kind="ExternalOutput").ap(),
    }
    with ExitStack() as ctx:
        tc = ctx.enter_context(tile.TileContext(nc))
        emit_block(ctx, tc, outs, ins, ntok, zflags)
    nc.finalize()
    return nc


def kernel(**inputs):
    x = np.ascontiguousarray(np.asarray(inputs["x"], np.float32))
    consts = preprocess(inputs)
    nc = build(NTOK_FULL)
    xs = x.reshape(NCORES, NTOK_FULL, C)
    in_maps = [dict(consts, x=np.ascontiguousarray(xs[c])) for c in range(NCORES)]
    res = run_bass_kernel_spmd(nc, in_maps, core_ids=list(range(NCORES)))
    out = np.stack([res.results[c]["out"] for c in range(NCORES)], axis=0)
    return out.reshape(B, T, C).astype(np.float32)


if __name__ == "__main__":
    rng = np.random.default_rng(0)
    fake = {
        "x": rng.standard_normal((B, T, C), dtype=np.float32),
        "ln1_g": np.ones(C, np.float32), "ln1_b": np.zeros(C, np.float32),
        "wq": rng.standard_normal((H, C, D), dtype=np.float32) * 0.02,
        "wk": rng.standard_normal((H, C, D), dtype=np.float32) * 0.02,
        "wv": rng.standard_normal((H, C, D), dtype=np.float32) * 0.02,
        "w_proj": rng.standard_normal((HD, C), dtype=np.float32) * 0.02,
        "b_proj": np.zeros(C, np.float32),
        "ln2_g": np.ones(C, np.float32), "ln2_b": np.zeros(C, np.float32),
        "w1": rng.standard_normal((C, M1), dtype=np.float32) * 0.02,
        "b1": np.zeros(M1, np.float32),
        "w2": rng.standard_normal((M1, C), dtype=np.float32) * 0.02,
        "b2": np.zeros(C, np.float32),
    }
    out = kernel(**fake)
    print("kernel ran, out shape", out.shape)


# revision 18
# speedup vs baseline: 1.2111x; 1.0954x over previous
"""Trainium2 Bass/Tile kernel for a dense transformer block.

Math (per batch element b, T=16 tokens, C=512, H=8 heads, D=64):
    h  = LN(x; ln1_g, ln1_b)
    q,k,v = per-head projections of h
    att = causal-softmax(q k^T / sqrt(D)); o = att v (heads concatenated)
    y  = o @ w_proj + b_proj + x
    f  = relu(LN(y; ln2_g, ln2_b) @ w1 + b1) @ w2 + b2
    out = f + y

Distribution: pure data parallel over the batch dim (4096) across 8
NeuronCores; weights replicated; no collectives.

v4: fully cyclic software pipeline over 128-token tiles; every
iteration emits the same work mix (v+logits+softmax for tile ti,
o-matmuls for ti-1, proj+LN2-stats for ti-2, LN2-apply for ti-3, LN1
stats/apply for tiles 4-5 ahead, plus a steady drip of deferred MLP
chunks) so the PE never idles long enough for the HAM clock gate to
re-throttle.
  * causal mask stays additive on the PE (accumulating ident x mask
    matmul into the logits PSUM) -- off the DVE softmax chain.
  * the two exp halves are one [128,1024] activation over a 2-bank
    PSUM tile.
  * att row-scale is one stride-0-broadcast tensor_tensor.
  * LN stats (bn_stats/aggr + the two tiny rstd activations) run 1-5
    iterations ahead of their apply, so the tiny scalar ops never
    block urgent scalar work (exp / evictions) in the FIFO.
  * c2 (MLP down-proj) emits token-major (lhsT = r slices, rhs = w2
    row-paired fp8 DR): no fT transposes; eviction fused with +y.
  * proj runs fp8 DoubleRow (oT evicted WS-scaled fp8); q,k fp8.
  * rank-1 bias matmuls (bpe, b2) are skipped at build time when the
    folded biases are exactly zero (true for this model's inputs).
"""

import sys

sys.path.insert(0, "/opt/trn_rl_repo")

import numpy as np
import ml_dtypes

import concourse.bass as bass
import concourse.tile as tile
from concourse import bacc, mybir
from concourse.bass import broadcast_tensor_aps
from concourse.bass_utils import run_bass_kernel_spmd

F32 = mybir.dt.float32
BF16 = mybir.dt.bfloat16
FP8 = mybir.dt.float8e4
DR = mybir.MatmulPerfMode.DoubleRow
AF = mybir.ActivationFunctionType
ALU = mybir.AluOpType

WS = 32.0

NCORES = 8
B, T, C, H, D = 4096, 16, 512, 8, 64
HD = H * D          # 512
M1 = 4 * C          # 2048
EPS = 1e-5
BL = B // NCORES    # 512 batch elems per core
NTOK_FULL = BL * T  # 8192 tokens per core
P = 128             # partitions
GT = 512            # tokens per group
KC = C // P         # 4 c-chunks
KM = M1 // P        # 16 hidden chunks

# deferred-MLP drain counts by tile phase (chunks for group g are created
# at ti=4g+6; phase 0 is light because B1 runs there)
DRAINS = (2, 6, 6, 6)

# set by preprocess(): which folded biases are exactly zero (lets build()
# skip the rank-1 bias matmuls; correctness-neutral, they are just zeros)
_ZFLAGS = {"bpe": False, "b2": False}


def emit_block(ctx, tc, outs, ins, ntok, zflags):
    """Emit the transformer-block program. outs/ins: dicts of DRAM APs."""
    nc = tc.nc
    x_d = ins["x"]
    wqk_d = ins["wqk"]
    wv_d = ins["wv"]
    wp_d = ins["wp"]
    w1_d = ins["w1"]
    w2_d = ins["w2"]
    out_d = outs["out"]

    ngroups = ntok // GT
    ntiles = ntok // P
    assert ntok % GT == 0

    consts = ctx.enter_context(tc.tile_pool(name="consts", bufs=1))

    wqk = [consts.tile([P, 2 * 2 * HD], FP8, tag=f"wqk{m}", name=f"wqk{m}")
           for m in range(2)]
    wv = [consts.tile([P, 2 * HD], FP8, tag=f"wv{m}", name=f"wv{m}")
          for m in range(2)]
    wp = [consts.tile([P, 2 * C], FP8, tag=f"wp{m}", name=f"wp{m}")
          for m in range(2)]
    w1 = [consts.tile([P, 2 * M1], FP8, tag=f"w1{m}", name=f"w1{m}")
          for m in range(2)]
    w2 = [consts.tile([P, 2 * C], FP8, tag=f"w2{m}", name=f"w2{m}")
          for m in range(8)]

    bqk = consts.tile([P, 12], F32, tag="bqk", name="bqk")
    scm = consts.tile([P, 2], F32, tag="scm", name="scm")
    b1t = consts.tile([P, KM], F32, tag="b1t", name="b1t")
    nc.sync.dma_start(bqk[:], ins["bqk"][:, :])
    nc.sync.dma_start(scm[:], ins["scm"][:, :])
    nc.sync.dma_start(b1t[:], ins["b1"].rearrange("(j p) -> p j", p=P))

    if not zflags["bpe"]:
        bpe = consts.tile([1, C], BF16, tag="bpe", name="bpe")
        nc.sync.dma_start(bpe[:], ins["bpe"][:, :])
    if not zflags["b2"]:
        b2r = consts.tile([1, C], BF16, tag="b2r", name="b2r")
        nc.sync.dma_start(b2r[:], ins["b2r"][:, :])
    # additive causal mask, 0 on allowed block-diag positions and
    # -300*WS^2 elsewhere (the exp scale is 0.125/WS^2)
    mask = consts.tile([P, KC * P], BF16, tag="mask", name="mask")
    nc.sync.dma_start(mask[:], ins["mask"][:, :])
    ident = consts.tile([P, P], BF16, tag="ident", name="ident")
    nc.sync.dma_start(ident[:], ins["ident"][:, :])
    ones1 = consts.tile([1, P], BF16, tag="ones1", name="ones1")
    nc.vector.memset(ones1[:], 1.0)
    epst = consts.tile([P, 1], F32, tag="epst", name="epst")
    nc.vector.memset(epst[:], EPS)

    def dma_weights_front():
        """Weights needed first (B1 / v of tile 0)."""
        for m in range(2):
            nc.sync.dma_start(wqk[m][:], wqk_d[m * P:(m + 1) * P, :])
            nc.sync.dma_start(wv[m][:], wv_d[m * P:(m + 1) * P, :])

    def dma_weights_rest():
        """Weights first needed several iterations in (b4 / MLP chunks)."""
        for m in range(2):
            nc.sync.dma_start(wp[m][:], wp_d[m * P:(m + 1) * P, :])
            nc.sync.dma_start(w1[m][:], w1_d[m * P:(m + 1) * P, :])
        for m in range(8):
            nc.sync.dma_start(w2[m][:], w2_d[m * P:(m + 1) * P, :])

    # --- working pools ---
    p_x = ctx.enter_context(tc.tile_pool(name="p_x", bufs=14))
    p_h = ctx.enter_context(tc.tile_pool(name="p_h", bufs=3))
    p_hT = ctx.enter_context(tc.tile_pool(name="p_hT", bufs=2))
    p_qk = ctx.enter_context(tc.tile_pool(name="p_qk", bufs=24))
    p_v = ctx.enter_context(tc.tile_pool(name="p_v", bufs=6))
    p_S = ctx.enter_context(tc.tile_pool(name="p_S", bufs=4))
    p_attT = ctx.enter_context(tc.tile_pool(name="p_attT", bufs=4))
    p_oT = ctx.enter_context(tc.tile_pool(name="p_oT", bufs=6))
    p_y = ctx.enter_context(tc.tile_pool(name="p_y", bufs=10))
    p_h2T = ctx.enter_context(tc.tile_pool(name="p_h2T", bufs=3))
    p_r = ctx.enter_context(tc.tile_pool(name="p_r", bufs=20))
    p_out = ctx.enter_context(tc.tile_pool(name="p_out", bufs=4))
    p_st = ctx.enter_context(tc.tile_pool(name="p_st", bufs=12))

    ps_mm = ctx.enter_context(tc.tile_pool(name="ps_mm", bufs=4, space="PSUM"))
    ps_log = ctx.enter_context(tc.tile_pool(name="ps_log", bufs=1, space="PSUM"))
    ps_t = ctx.enter_context(tc.tile_pool(name="ps_t", bufs=2, space="PSUM"))

    # --- cross-iteration state ---
    xs = {}        # tj -> x tile
    hTs = {}       # g -> hT group tile
    qTs = {}       # g -> (qT, kTe, kTo) lists
    vs = {}        # tj -> v tile
    oTs = {}       # tj -> oT tile
    attTs = {}     # tj -> attT tile
    ys = {}        # tj -> y tile
    h2Ts = {}      # g -> h2T group tile
    ln1s = {}      # tj -> (mv, rstd)
    ln2s = {}      # tj -> (mv, rstd)
    pend_c = []

    def drain_c(n):
        for _ in range(min(n, len(pend_c))):
            pend_c.pop(0)()

    def ln_stats(src, store, tj):
        st = p_st.tile([P, 6], F32, tag="bn", name="bn")
        mv = p_st.tile([P, 2], F32, tag="mv", name="mv")
        nc.vector.bn_stats(st[:], src[:])
        nc.vector.bn_aggr(mv[:], st[:])
        lnv = p_st.tile([P, 1], F32, tag="lnv", name="lnv")
        rstd = p_st.tile([P, 1], F32, tag="rstd", name="rstd")
        nc.scalar.activation(lnv[:], mv[:, 1:2], AF.Ln, bias=epst[:])
        nc.scalar.activation(rstd[:], lnv[:], AF.Exp, scale=-0.5)
        store[tj] = (mv, rstd)

    def ln_apply(src, store, tj, h_t):
        mv, rstd = store.pop(tj)
        nc.vector.tensor_scalar(
            out=h_t[:], in0=src[:],
            scalar1=mv[:, 0:1], scalar2=rstd[:],
            op0=ALU.subtract, op1=ALU.mult,
        )

    def stage_a_dma(g):
        for i in range(4):
            tj = 4 * g + i
            xt = p_x.tile([P, C], F32, tag="x", name="x")
            xs[tj] = xt
            nc.sync.dma_start(xt[:], x_d[tj * P:(tj + 1) * P, :])

    def ln1_stats(tj):
        ln_stats(xs[tj], ln1s, tj)

    def ln1_apply(tj):
        """LN1 apply + feature-major transpose of tile tj into hTs[g]."""
        g, i = divmod(tj, 4)
        if i == 0:
            hTs[g] = p_hT.tile([P, KC * GT], FP8, tag="hT", name="hT")
        hT3 = hTs[g][:].rearrange("p (c t) -> p c t", c=KC)
        ht = p_h.tile([P, C], BF16, tag="h", name="h")
        ln_apply(xs[tj], ln1s, tj, ht)
        # transpose via regular matmul against identity (out = ht_c^T):
        # runs at the warm 2.4 GHz PE clock and counts as HAM activity,
        # unlike transpose-mode (fixed 1.2 GHz, HAM-invisible).
        pst = ps_t.tile([P, KC * P], F32, tag="pst", name="pst")
        for c in range(KC):
            nc.tensor.matmul(pst[:, c * P:(c + 1) * P],
                             ht[:, c * P:(c + 1) * P], ident[:],
                             start=True, stop=True)
        nc.vector.tensor_copy(hT3[:, :, i * P:(i + 1) * P], pst[:])

    def emit_b1(g):
        """q^T, k^T for group g. k is evicted twice per chunk with 0/1
        per-partition scale masks (zero-padded single-head copies)."""
        hT3 = hTs[g][:].rearrange("p (c t) -> p c t", c=KC)
        qT = [p_qk.tile([P, GT], FP8, tag="qk", name="qk") for _ in range(KC)]
        kTe = [p_qk.tile([P, GT], FP8, tag="qk", name="qk") for _ in range(KC)]
        kTo = [p_qk.tile([P, GT], FP8, tag="qk", name="qk") for _ in range(KC)]
        qTs[g] = (qT, kTe, kTo)
        for j in range(8):
            ps = ps_mm.tile([P, GT], F32, tag="mm", name="mm")
            for m in range(2):
                wqks = wqk[m][:].rearrange("p (c j) -> p c j", c=2)
                nc.tensor.matmul(
                    ps[:], wqks[:, :, j * P:(j + 1) * P],
                    hT3[:, 2 * m:2 * m + 2, :],
                    start=(m == 0), stop=(m == 1), perf_mode=DR,
                )
            if j < 4:
                nc.scalar.activation(qT[j][:], ps[:], AF.Identity,
                                     bias=bqk[:, j:j + 1])
            else:
                hp = j - 4
                nc.scalar.activation(kTe[hp][:], ps[:], AF.Identity,
                                     scale=scm[:, 0:1], bias=bqk[:, 4 + hp:5 + hp])
                nc.scalar.activation(kTo[hp][:], ps[:], AF.Identity,
                                     scale=scm[:, 1:2], bias=bqk[:, 8 + hp:9 + hp])

    def emit_v(tj):
        g, i = divmod(tj, 4)
        hT3 = hTs[g][:].rearrange("p (c t) -> p c t", c=KC)
        ps = ps_mm.tile([P, HD], F32, tag="mm", name="mm")
        for m in range(2):
            wvs = wv[m][:].rearrange("p (c d) -> p c d", c=2)
            nc.tensor.matmul(
                ps[:], hT3[:, 2 * m:2 * m + 2, i * P:(i + 1) * P],
                wvs[:],
                start=(m == 0), stop=(m == 1), perf_mode=DR,
            )
        vt = p_v.tile([P, HD], BF16, tag="v", name="v")
        vs[tj] = vt
        nc.scalar.activation(vt[:], ps[:], AF.Identity, scale=1.0 / WS)

    def emit_att(tj):
        """logits + additive mask (PE) + one merged exp + normalize + attT."""
        g, i = divmod(tj, 4)
        qT, kTe, kTo = qTs[g]
        sl = slice(i * P, (i + 1) * P)
        ps_l = ps_log.tile([P, 2 * C], F32, tag="log", name="log")
        for half in range(2):
            hof = half * C
            for hh in range(4):
                h = half * 4 + hh
                hp = h // 2
                kk = kTe[hp] if h % 2 == 0 else kTo[hp]
                nc.tensor.matmul(
                    ps_l[:, hof + hh * P:hof + (hh + 1) * P],
                    qT[hp][:, sl], kk[:, sl],
                    start=(hh == 0), stop=False,
                )
            nc.tensor.matmul(ps_l[:, hof:hof + C], ident[:], mask[:],
                             start=False, stop=True)
        S = p_S.tile([P, H * P], BF16, tag="S", name="S")
        nc.scalar.activation(S[:], ps_l[:], AF.Exp,
                             scale=float(D) ** -0.5 / (WS * WS))
        S3 = S[:].rearrange("p (h s) -> p h s", h=H)
        rs = p_st.tile([P, H], F32, tag="rs", name="rs")
        nc.vector.tensor_reduce(
            out=rs[:], in_=S3,
            axis=mybir.AxisListType.X, op=ALU.add,
        )
        rr = p_st.tile([P, H], F32, tag="rr", name="rr")
        nc.vector.reciprocal(rr[:], rs[:])
        att = p_attT.tile([P, H * P], BF16, tag="attbuf", name="attbuf")
        att3 = att[:].rearrange("p (h s) -> p h s", h=H)
        r3 = rr[:].rearrange("p (h o) -> p h o", o=1)
        _, r_bc = broadcast_tensor_aps(S3, r3)
        nc.vector.tensor_tensor(out=att3, in0=S3, in1=r_bc, op=ALU.mult)
        attT = p_attT.tile([P, H * P], BF16, tag="attT", name="attT")
        nc.vector.transpose(attT[:], att[:])
        attTs[tj] = attT

    def emit_oT(tj):
        attT = attTs.pop(tj)
        vt = vs.pop(tj)
        ps_o = ps_mm.tile([P, C], F32, tag="mm", name="mm")
        for h in range(H):
            nc.tensor.matmul(
                ps_o[(h % 2) * 64:(h % 2) * 64 + 64,
                     (h // 2) * P:(h // 2) * P + P],
                vt[:, h * 64:(h + 1) * 64],
                attT[:, h * P:(h + 1) * P],
                start=True, stop=True,
            )
        ot = p_oT.tile([P, C], FP8, tag="oT", name="oT")
        oTs[tj] = ot
        nc.scalar.activation(ot[:], ps_o[:], AF.Identity, scale=WS)

    def emit_b4(tj):
        ot = oTs.pop(tj)
        xt = xs.pop(tj)
        ps = ps_mm.tile([P, C], F32, tag="mm", name="mm")
        oT4 = ot[:].rearrange("p (m c t) -> p m c t", m=2, c=2)
        for m in range(2):
            wps = wp[m][:].rearrange("p (c n) -> p c n", c=2)
            nc.tensor.matmul(
                ps[:], oT4[:, m], wps[:],
                start=(m == 0), stop=(m == 1 and zflags["bpe"]), perf_mode=DR,
            )
        if not zflags["bpe"]:
            nc.tensor.matmul(ps[:], ones1[:], bpe[:], start=False, stop=True)
        yt = p_y.tile([P, C], F32, tag="y", name="y")
        ys[tj] = yt
        nc.vector.scalar_tensor_tensor(
            out=yt[:], in0=ps[:], scalar=1.0 / (WS * WS), in1=xt[:],
            op0=ALU.mult, op1=ALU.add,
        )
        # LN2 stats run here, one iteration ahead of the apply, so the
        # tiny rstd activations never block urgent scalar work.
        ln_stats(yt, ln2s, tj)

    def emit_b5(tj):
        g, i = divmod(tj, 4)
        if i == 0:
            h2Ts[g] = p_h2T.tile([P, KC * GT], FP8, tag="h2T", name="h2T")
        h2T3 = h2Ts[g][:].rearrange("p (c t) -> p c t", c=KC)
        ht2 = p_h.tile([P, C], BF16, tag="h", name="h")
        ln_apply(ys[tj], ln2s, tj, ht2)
        pst = ps_t.tile([P, KC * P], F32, tag="pst", name="pst")
        for c in range(KC):
            nc.tensor.matmul(pst[:, c * P:(c + 1) * P],
                             ht2[:, c * P:(c + 1) * P], ident[:],
                             start=True, stop=True)
        nc.vector.tensor_copy(h2T3[:, :, i * P:(i + 1) * P], pst[:])

    def make_c_chunks(g):
        """Deferred MLP of group g as emission closures."""
        h2Tg = h2Ts[g]
        r_t = []
        chunks = []
        h2T3 = h2Tg[:].rearrange("p (c t) -> p c t", c=KC)

        def c1(j):
            ps = ps_mm.tile([P, GT], F32, tag="mm", name="mm")
            for m in range(2):
                w1s = w1[m][:].rearrange("p (c j) -> p c j", c=2)
                nc.tensor.matmul(
                    ps[:], w1s[:, :, j * P:(j + 1) * P],
                    h2T3[:, 2 * m:2 * m + 2, :],
                    start=(m == 0), stop=(m == 1), perf_mode=DR,
                )
            if j % 2 == 0:
                r_t.append(p_r.tile([P, 2 * GT], FP8, tag="r", name="r"))
            rt = r_t[j // 2]
            nc.scalar.activation(rt[:, (j % 2) * GT:(j % 2 + 1) * GT], ps[:],
                                 AF.Relu, bias=b1t[:, j:j + 1])

        def c2t(i):
            tj = 4 * g + i
            ps = ps_mm.tile([P, C], F32, tag="mm", name="mm")
            for m in range(8):
                r3 = r_t[m][:].rearrange("p (c t) -> p c t", c=2)
                w2s = w2[m][:].rearrange("p (c n) -> p c n", c=2)
                nc.tensor.matmul(
                    ps[:], r3[:, :, i * P:(i + 1) * P], w2s[:],
                    start=(m == 0), stop=(m == 7 and zflags["b2"]),
                    perf_mode=DR,
                )
            if not zflags["b2"]:
                nc.tensor.matmul(ps[:], ones1[:], b2r[:], start=False,
                                 stop=True)
            yt = ys.pop(tj)
            ot = p_out.tile([P, C], F32, tag="out", name="out")
            nc.vector.scalar_tensor_tensor(
                out=ot[:], in0=ps[:], scalar=1.0 / (WS * WS), in1=yt[:],
                op0=ALU.mult, op1=ALU.add,
            )
            nc.sync.dma_start(out_d[tj * P:(tj + 1) * P, :], ot[:])

        from functools import partial
        for j in range(KM):
            chunks.append(partial(c1, j))
        for i in range(4):
            chunks.append(partial(c2t, i))
        return chunks

    # ---- prologue: group 0 (and group 1 stats head-start) ----
    # DMA order: x tiles + front weights first so compute starts early;
    # the bulk weights (wp/w1/w2, ~2MB) follow and overlap the first
    # iterations (they are not needed until b4 / the MLP chunks).
    stage_a_dma(0)
    stage_a_dma(1)
    dma_weights_front()
    for tj in range(5):
        ln1_stats(tj)
    for tj in range(4):
        ln1_apply(tj)
    dma_weights_rest()

    # ---- flat cyclic pipeline over tiles ----
    for ti in range(ntiles + 7):
        g, i = divmod(ti, 4)
        if ti < ntiles:
            if i == 0:
                emit_b1(g)
            emit_v(ti)
            emit_att(ti)
        if 0 <= ti - 1 < ntiles:
            emit_oT(ti - 1)
        if 0 <= ti - 2 < ntiles:
            emit_b4(ti - 2)
        drain_c(DRAINS[i] if ti < ntiles else 1000)
        if 0 <= ti - 3 < ntiles:
            emit_b5(ti - 3)
            if (ti - 3) % 4 == 3:
                gc = (ti - 3) // 4
                pend_c.extend(make_c_chunks(gc))
                del h2Ts[gc]
        if ti < ntiles:
            if i == 2 and g + 2 < ngroups:
                stage_a_dma(g + 2)
            if ti + 5 < ntiles:
                ln1_stats(ti + 5)
            if ti + 4 < ntiles:
                ln1_apply(ti + 4)
        if i == 3 and g - 1 >= 0:
            qTs.pop(g - 1, None)
            hTs.pop(g - 1, None)

    drain_c(len(pend_c))


def _fp8_pair(w, ncols):
    """[K, ncols] f32 -> [K//256, 128, 2, ncols] fp8 paired chunks, flattened
    to [K//2, 2*ncols] for a contiguous per-tile DMA."""
    fp8 = ml_dtypes.float8_e4m3
    K = w.shape[0]
    npair = K // 256
    wp = w.reshape(npair, 2, P, ncols).transpose(0, 2, 1, 3)  # [m, p, c, cols]
    wp = np.clip(wp * WS, -240, 240)
    return np.ascontiguousarray(wp.reshape(npair * P, 2 * ncols)).astype(fp8)


def preprocess(inputs):
    """Host-side weight folding. Returns dict of extra device arrays."""
    f32 = np.float32
    bf16 = ml_dtypes.bfloat16
    fp8 = ml_dtypes.float8_e4m3
    g1 = np.asarray(inputs["ln1_g"], f32)
    b1n = np.asarray(inputs["ln1_b"], f32)
    g2 = np.asarray(inputs["ln2_g"], f32)
    b2n = np.asarray(inputs["ln2_b"], f32)
    wq = np.asarray(inputs["wq"], f32).transpose(1, 0, 2).reshape(C, HD)
    wk = np.asarray(inputs["wk"], f32).transpose(1, 0, 2).reshape(C, HD)
    wv = np.asarray(inputs["wv"], f32).transpose(1, 0, 2).reshape(C, HD)
    w_proj = np.asarray(inputs["w_proj"], f32)
    b_proj = np.asarray(inputs["b_proj"], f32)
    w1 = np.asarray(inputs["w1"], f32)
    b1 = np.asarray(inputs["b1"], f32)
    w2 = np.asarray(inputs["w2"], f32)
    b2 = np.asarray(inputs["b2"], f32)

    wqg = wq * g1[:, None]
    wkg = wk * g1[:, None]
    wvg = wv * g1[:, None]
    w1g = w1 * g2[:, None]

    wqk = np.concatenate([wqg, wkg], axis=1)          # [C, 1024]
    bqk = b1n @ wqk                                   # [1024]
    bias_v = b1n @ wvg                                # [512]
    bpe = (bias_v @ w_proj + b_proj)[None, :]         # [1, 512]
    bias1 = b2n @ w1g + b1                            # [2048]

    _ZFLAGS["bpe"] = bool(np.all(bpe == 0.0))
    _ZFLAGS["b2"] = bool(np.all(b2 == 0.0))

    # w_proj packed for fp8 DR with oT's (d-fold, head-pair) layout.
    wp4 = w_proj.reshape(H, D, C)
    wpp = np.zeros((2, P, 2, C), f32)
    for m in range(2):
        for e in range(2):
            for c in range(2):
                wpp[m, 64 * e:64 * (e + 1), c, :] = wp4[2 * (2 * m + c) + e]
    wpp = np.clip(wpp * WS, -240, 240).reshape(2 * P, 2 * C)

    # additive mask: 0 on allowed (block-diag causal), -300*WS^2 elsewhere
    mask = np.full((P, P), -300.0 * WS * WS, f32)
    tril = np.tril(np.ones((T, T), bool))
    for b in range(P // T):
        blk = mask[b * T:(b + 1) * T, b * T:(b + 1) * T]
        blk[tril] = 0.0

    # bqk [128, 12]: cols 0-3 q biases, 4-7 k biases (odd rows zeroed),
    # 8-11 k biases (even rows zeroed); scm holds the 0/1 row masks.
    bqkw = bqk * WS
    bqkx = np.zeros((P, 12), f32)
    for j in range(4):
        bqkx[:, j] = bqkw[j * P:(j + 1) * P]
        kcol = bqkw[HD + j * P:HD + (j + 1) * P]
        bqkx[:64, 4 + j] = kcol[:64]
        bqkx[64:, 8 + j] = kcol[64:]
    scm = np.zeros((P, 2), f32)
    scm[:64, 0] = 1.0
    scm[64:, 1] = 1.0

    return {
        "wqk": _fp8_pair(wqk, 2 * HD),
        "wv": _fp8_pair(wvg, HD),
        "wp": wpp.astype(fp8),
        "w1": _fp8_pair(w1g, M1),
        "w2": _fp8_pair(w2, C),
        "bqk": bqkx,
        "scm": scm,
        "b1": (bias1 * WS).astype(f32),
        "b2r": (b2 * WS * WS)[None, :].astype(bf16),
        "bpe": (bpe * WS * WS).astype(bf16),
        "mask": np.tile(mask, (1, KC)).astype(bf16),
        "ident": np.eye(P, dtype=bf16),
    }


def _patch_act_tables():
    """Make every activation func we use resolve to the single table set
    `natural_log_exp_and_others` (it contains Ln, Exp, Identity and Relu),
    so bacc's table-load pass emits one load instead of thrashing between
    `natural_log` and `exp_and_others` (~2.7us per switch)."""
    import concourse.bacc as _bacc_mod
    import concourse.hw_specs as _hw
    if getattr(_bacc_mod, "_ant_act_tables_patched", False):
        return
    _orig = _hw.get_activation_tables
    ours = {AF.Ln, AF.Exp, AF.Identity, AF.Relu, AF.Copy}

    def patched(arch):
        tables = _orig(arch)
        out = {}
        for name, funcs in tables.items():
            if name == "natural_log_exp_and_others":
                out[name] = funcs
            else:
                out[name] = funcs - ours
        return out

    _bacc_mod.get_activation_tables = patched
    _bacc_mod._ant_act_tables_patched = True


def build(ntok=NTOK_FULL, zflags=None):
    """Build the Bass program; returns nc."""
    from contextlib import ExitStack

    if zflags is None:
        zflags = dict(_ZFLAGS)
    _patch_act_tables()
    nc = bacc.Bacc("TRN2", target_bir_lowering=False, debug=False,
                   enable_asserts=False, num_devices=NCORES)
    ins = {
        "x": nc.dram_tensor("x", [ntok, C], F32, kind="ExternalInput").ap(),
        "wqk": nc.dram_tensor("wqk", [2 * P, 2 * 2 * HD], FP8,
                              kind="ExternalInput").ap(),
        "wv": nc.dram_tensor("wv", [2 * P, 2 * HD], FP8,
                             kind="ExternalInput").ap(),
        "wp": nc.dram_tensor("wp", [2 * P, 2 * C], FP8,
                             kind="ExternalInput").ap(),
        "w1": nc.dram_tensor("w1", [2 * P, 2 * M1], FP8,
                             kind="ExternalInput").ap(),
        "w2": nc.dram_tensor("w2", [8 * P, 2 * C], FP8,
                             kind="ExternalInput").ap(),
        "bqk": nc.dram_tensor("bqk", [P, 12], F32, kind="ExternalInput").ap(),
        "scm": nc.dram_tensor("scm", [P, 2], F32, kind="ExternalInput").ap(),
        "b1": nc.dram_tensor("b1", [M1], F32, kind="ExternalInput").ap(),
        "b2r": nc.dram_tensor("b2r", [1, C], BF16, kind="ExternalInput").ap(),
        "bpe": nc.dram_tensor("bpe", [1, C], BF16, kind="ExternalInput").ap(),
        "mask": nc.dram_tensor("mask", [P, KC * P], BF16,
                               kind="ExternalInput").ap(),
        "ident": nc.dram_tensor("ident", [P, P], BF16,
                                kind="ExternalInput").ap(),
    }
    outs = {
        "out": nc.dram_tensor("out", [ntok, C], F32, kind="ExternalOutput").ap(),
    }
    with ExitStack() as ctx:
        tc = ctx.enter_context(tile.TileContext(nc))
        emit_block(ctx, tc, outs, ins, ntok, zflags)
    nc.finalize()
    return nc


def kernel(**inputs):
    x = np.ascontiguousarray(np.asarray(inputs["x"], np.float32))
    consts = preprocess(inputs)
    nc = build(NTOK_FULL)
    xs = x.reshape(NCORES, NTOK_FULL, C)
    in_maps = [dict(consts, x=np.ascontiguousarray(xs[c])) for c in range(NCORES)]
    res = run_bass_kernel_spmd(nc, in_maps, core_ids=list(range(NCORES)))
    out = np.stack([res.results[c]["out"] for c in range(NCORES)], axis=0)
    return out.reshape(B, T, C).astype(np.float32)


if __name__ == "__main__":
    rng = np.random.default_rng(0)
    fake = {
        "x": rng.standard_normal((B, T, C), dtype=np.float32),
        "ln1_g": np.ones(C, np.float32), "ln1_b": np.zeros(C, np.float32),
        "wq": rng.standard_normal((H, C, D), dtype=np.float32) * 0.02,
        "wk": rng.standard_normal((H, C, D), dtype=np.float32) * 0.02,
        "wv": rng.standard_normal((H, C, D), dtype=np.float32) * 0.02,
        "w_proj": rng.standard_normal((HD, C), dtype=np.float32) * 0.02,
        "b_proj": np.zeros(C, np.float32),
        "ln2_g": np.ones(C, np.float32), "ln2_b": np.zeros(C, np.float32),
        "w1": rng.standard_normal((C, M1), dtype=np.float32) * 0.02,
        "b1": np.zeros(M1, np.float32),
        "w2": rng.standard_normal((M1, C), dtype=np.float32) * 0.02,
        "b2": np.zeros(C, np.float32),
    }
    out = kernel(**fake)
    print("kernel ran, out shape", out.shape)
